# revision 1
# baseline (speedup 1.0000x reference)
"""Trainium2 Bass kernel for nn_Equalize (soft histogram equalization).

Algorithm (per core; 8 cores, each owns a quarter of one of the 2 images):
  1. Fine histogram (8160 bins) of the core's 65536 pixels via two-level
     one-hot (96 x 85) outer-product matmuls accumulated in PSUM.
  2. AllReduce the fine histogram across the 4 cores of each image.
  3. Coarse 256-bin soft histogram = Toeplitz window-conv of the fine
     histogram with the Gaussian kernel (DVE mul+reduce on strided DMA views).
  4. cdf via triangular matmul; normalize to cdfn.
  5. G lookup table (4096 entries): G(v) = sum_j k(v-b_j)*cdfn[j] / sum_j k(v-b_j)
     computed with a small Toeplitz matmul (window of 32 bins).
  6. Per-pixel output = G[round(x*4080)] via GPSIMD ap_gather.

The output of the reference only depends on a pixel through the smooth 1-D
function G, so a fine table lookup reproduces it to ~1e-4.
"""
import os
import numpy as np

import concourse.bass as bass
import concourse.mybir as mybir
import concourse.tile as tile
import concourse.bacc as bacc
from concourse.bass_utils import run_bass_kernel_spmd

F32 = mybir.dt.float32
I32 = mybir.dt.int32
I16 = mybir.dt.int16
BF16 = mybir.dt.bfloat16

B, H, W = 2, 512, 512
N_CORES = 8
QUARTER = H // 4 * W            # 65536 pixels per core
N_BINS = 256
TAU = 0.01
C = 1.0 / (2.0 * TAU * TAU)     # 5000
SQC = float(np.sqrt(C))
NF = 8160                        # fine-hist resolution (32*255)
NHI, NLO = 96, 85                # NF = NHI*NLO
TWIN = 1024                      # conv window (fine bins)
PADL = TWIN // 2                 # 512
HF_LEN = PADL + NF + (TWIN // 2 + 32)   # 9216, padded fine hist
TGRID = 4080                     # G-table grid (16*255)
MSUB = 16                        # table sub-samples per bin
TLEN = 4096                      # table allocation (num_elems)
KWIN = 32                        # G window in coarse bins
NTILE = QUARTER // 128           # 512 pixel tiles
NPX_GRP = QUARTER // 8           # 8192 pixels per gpsimd core group


def mk_ap(handle_ap, offset, pairs):
    import dataclasses
    return dataclasses.replace(handle_ap, offset=offset, ap=list(pairs))


def build_nc(stage=3):
    stage = int(os.environ.get("KERNEL_STAGE", stage))
    nc = bacc.Bacc()
    x_dram = nc.declare_dram_parameter("x", [QUARTER], F32, isOutput=False)
    out_dram = nc.declare_dram_parameter("out", [QUARTER], F32, isOutput=True)

    hf_dram = nc.dram_tensor("hf_local", [HF_LEN], F32)
    hf_red = nc.dram_tensor("hf_red", [HF_LEN], F32)
    cpad_dram = nc.dram_tensor("cpad", [N_BINS + KWIN], F32)    # 288
    vpad_dram = nc.dram_tensor("vpad", [N_BINS + KWIN], F32)
    gtab_dram = nc.dram_tensor("gtab", [TLEN], F32)

    with tile.TileContext(nc) as tc:
        with (
            tc.tile_pool(name="big", bufs=1) as big,
            tc.tile_pool(name="oh", bufs=4) as ohp,
            tc.tile_pool(name="small", bufs=1) as sm,
            tc.tile_pool(name="psum", bufs=1, space="PSUM") as psp,
        ):
            # ---------------- constants ----------------
            iota_hi_i = sm.tile([128, NHI], I32)
            nc.gpsimd.iota(iota_hi_i[:], pattern=[[1, NHI]], base=0, channel_multiplier=0)
            iota_hi = sm.tile([128, NHI], BF16)
            nc.vector.tensor_copy(iota_hi[:], iota_hi_i[:])

            iota_lo_i = sm.tile([128, NLO], I32)
            nc.gpsimd.iota(iota_lo_i[:], pattern=[[1, NLO]], base=0, channel_multiplier=0)
            iota_lo = sm.tile([128, NLO], BF16)
            nc.vector.tensor_copy(iota_lo[:], iota_lo_i[:])

            # kw[t] = exp(-C*((t-511.5)/NF)^2), replicated per partition
            kw_i = sm.tile([128, TWIN], I32)
            nc.gpsimd.iota(kw_i[:], pattern=[[1, TWIN]], base=0, channel_multiplier=0)
            kw_f = sm.tile([128, TWIN], F32)
            nc.vector.tensor_copy(kw_f[:], kw_i[:])
            kw_sq = sm.tile([128, TWIN], F32)
            bias_kw = sm.tile([128, 1], F32)
            nc.vector.memset(bias_kw[:], -SQC * (TWIN / 2 - 0.5) / NF)
            nc.scalar.activation(kw_sq[:], kw_f[:], mybir.ActivationFunctionType.Square,
                                 bias=bias_kw[:], scale=SQC / NF)
            kw = sm.tile([128, TWIN], F32)
            nc.scalar.activation(kw[:], kw_sq[:], mybir.ActivationFunctionType.Exp,
                                 scale=-1.0)

            # W_win lhsT [32 k, 16 m]: exp(-C*((m + 256 - 16k)/TGRID)^2)
            ww_i = sm.tile([KWIN, MSUB], I32)
            nc.gpsimd.iota(ww_i[:], pattern=[[1, MSUB]], base=256, channel_multiplier=-16)
            ww_f = sm.tile([KWIN, MSUB], F32)
            nc.vector.tensor_copy(ww_f[:], ww_i[:])
            ww_sq = sm.tile([KWIN, MSUB], F32)
            bias_z32 = sm.tile([KWIN, 1], F32)
            nc.vector.memset(bias_z32[:], 0.0)
            nc.scalar.activation(ww_sq[:], ww_f[:], mybir.ActivationFunctionType.Square,
                                 bias=bias_z32[:], scale=SQC / TGRID)
            ww = sm.tile([KWIN, MSUB], F32)
            nc.scalar.activation(ww[:], ww_sq[:], mybir.ActivationFunctionType.Exp,
                                 scale=-1.0)

            # triangular matrices for cumsum: iota j-k
            tri_i = sm.tile([128, N_BINS], I16)
            nc.gpsimd.iota(tri_i[:], pattern=[[1, N_BINS]], base=0, channel_multiplier=-1)
            tri0 = sm.tile([128, N_BINS], F32)
            nc.vector.tensor_scalar(tri0[:], tri_i[:], 0.0, None, mybir.AluOpType.is_ge)
            tri1 = sm.tile([128, N_BINS], F32)
            nc.vector.tensor_scalar(tri1[:], tri_i[:], 128.0, None, mybir.AluOpType.is_ge)

            # zero row for DRAM padding; ones/valid row
            z_row = sm.tile([1, PADL + TWIN // 2 + 32], F32)
            nc.vector.memset(z_row[:], 0.0)
            vp_row = sm.tile([1, N_BINS + KWIN], F32)
            nc.vector.memset(vp_row[:], 0.0)
            nc.vector.memset(vp_row[:, KWIN // 2:KWIN // 2 + N_BINS], 1.0)
            nc.sync.dma_start(vpad_dram.ap(), vp_row[:])

            # ------------- early library warm-up for ap_gather -------------
            warm_tab = sm.tile([128, 4], F32)
            nc.vector.memset(warm_tab[:], 0.0)
            warm_idx = sm.tile([128, 1], I16)
            nc.vector.memset(warm_idx[:], 0)
            warm_out = sm.tile([128, 16], F32)
            nc.gpsimd.ap_gather(
                warm_out[:].rearrange("c (n d) -> c n d", d=1),
                warm_tab[:].rearrange("c (n d) -> c n d", d=1),
                warm_idx[:], channels=128, num_elems=4, d=1, num_idxs=16)

            # ---------------- phase 1: fine histogram ----------------
            x_sb = big.tile([128, NTILE], F32)
            nc.sync.dma_start(x_sb[:], x_dram.ap().rearrange("(p t) -> p t", p=128))

            # HW f32->int converts round-to-nearest-even, so floor(v) is
            # round(v - 0.5) (ties land on even, a half-fine-bin jitter).
            v_sb = big.tile([128, NTILE], F32)
            nc.vector.tensor_scalar(v_sb[:], x_sb[:], float(NHI), None, mybir.AluOpType.mult)
            hi_i = big.tile([128, NTILE], I32)
            nc.vector.tensor_scalar(hi_i[:], v_sb[:], 0.5, None, mybir.AluOpType.subtract)
            hi_f = big.tile([128, NTILE], F32)
            nc.vector.tensor_copy(hi_f[:], hi_i[:])
            fr_sb = big.tile([128, NTILE], F32)
            nc.vector.tensor_tensor(fr_sb[:], v_sb[:], hi_f[:], mybir.AluOpType.subtract)
            lo_i = big.tile([128, NTILE], I32)
            nc.vector.tensor_scalar(lo_i[:], fr_sb[:], float(NLO), 0.5,
                                    mybir.AluOpType.mult, mybir.AluOpType.subtract)
            lo_f = big.tile([128, NTILE], F32)
            nc.vector.tensor_copy(lo_f[:], lo_i[:])

            hf_psum = psp.tile([NHI, NLO], F32)
            for t in range(NTILE):
                oh_hi = ohp.tile([128, NHI], BF16)
                nc.vector.tensor_scalar(oh_hi[:], iota_hi[:], hi_f[:, t:t + 1], None,
                                        mybir.AluOpType.is_equal)
                oh_lo = ohp.tile([128, NLO], BF16)
                nc.vector.tensor_scalar(oh_lo[:], iota_lo[:], lo_f[:, t:t + 1], None,
                                        mybir.AluOpType.is_equal)
                nc.tensor.matmul(hf_psum[:], oh_hi[:], oh_lo[:],
                                 start=(t == 0), stop=(t == NTILE - 1))

            hf_sb = sm.tile([NHI, NLO], F32)
            nc.vector.tensor_copy(hf_sb[:], hf_psum[:])

            if stage == 1:
                nc.sync.dma_start(
                    out_dram.ap()[0:NF].rearrange("(a b) -> a b", a=NHI), hf_sb[:])
            else:
                # store padded fine hist to DRAM
                nc.sync.dma_start(hf_dram.ap()[0:PADL], z_row[:, 0:PADL])
                nc.sync.dma_start(hf_dram.ap()[PADL + NF:HF_LEN],
                                  z_row[:, 0:HF_LEN - PADL - NF])
                nc.sync.dma_start(
                    hf_dram.ap()[PADL:PADL + NF].rearrange("(a b) -> a b", a=NHI),
                    hf_sb[:])

                # ---------- allreduce over the 4 cores of this image ----------
                nc.gpsimd.collective_compute(
                    "AllReduce",
                    mybir.AluOpType.add,
                    ins=[hf_dram.ap().opt()],
                    outs=[hf_red.ap().opt()],
                    replica_groups=[[0, 1, 2, 3], [4, 5, 6, 7]],
                )

                if stage == 15:
                    hr_sb = sm.tile([NHI, NLO], F32)
                    nc.sync.dma_start(
                        hr_sb[:],
                        hf_red.ap()[PADL:PADL + NF].rearrange("(a b) -> a b", a=NHI))
                    nc.sync.dma_start(
                        out_dram.ap()[0:NF].rearrange("(a b) -> a b", a=NHI),
                        hr_sb[:])

                else:
                    # ---------- conv -> coarse hist (2 blocks of 128 bins) ----------
                    hist_cols = []
                    scr = sm.tile([128, TWIN], F32)
                    for blk in range(2):
                        hband = big.tile([128, TWIN], F32)
                        src = mk_ap(hf_red.ap(), 4096 * blk, [[32, 128], [1, TWIN]])
                        nc.sync.dma_start(hband[:], src)
                        hcol = sm.tile([128, 1], F32)
                        nc.vector.tensor_tensor(scr[:], hband[:], kw[:],
                                                mybir.AluOpType.mult)
                        nc.vector.tensor_reduce(hcol[:], scr[:],
                                                mybir.AxisListType.X,
                                                mybir.AluOpType.add)
                        hist_cols.append(hcol)

                    if stage == 17:
                        nc.sync.dma_start(
                            out_dram.ap()[0:128].rearrange("(a b) -> a b", a=128),
                            hist_cols[0][:])
                        nc.sync.dma_start(
                            out_dram.ap()[128:256].rearrange("(a b) -> a b", a=128),
                            hist_cols[1][:])

                    else:
                        # ---------- cdf via triangular matmul ----------
                        cdf_psum = psp.tile([1, N_BINS], F32)
                        nc.tensor.matmul(cdf_psum[:], hist_cols[0][:], tri0[:],
                                         start=True, stop=False)
                        nc.tensor.matmul(cdf_psum[:], hist_cols[1][:], tri1[:],
                                         start=False, stop=True)

                        if stage == 18:
                            cdump = sm.tile([1, N_BINS], F32)
                            nc.vector.tensor_copy(cdump[:], cdf_psum[:])
                            nc.sync.dma_start(
                                out_dram.ap()[0:N_BINS].rearrange("(a b) -> a b", a=1),
                                cdump[:])

                        else:
                            # cdfn = (cdf - cdf0) / (cdf_end - cdf0)
                            cdf_sb = sm.tile([1, N_BINS], F32)
                            nc.vector.tensor_copy(cdf_sb[:], cdf_psum[:])
                            c0 = cdf_sb[:, 0:1]
                            cend = cdf_sb[:, N_BINS - 1:N_BINS]
                            denom = sm.tile([1, 1], F32)
                            nc.vector.tensor_tensor(denom[:], cend, c0, mybir.AluOpType.subtract)
                            rden = sm.tile([1, 1], F32)
                            nc.vector.reciprocal(rden[:], denom[:])
                            cp_row = sm.tile([1, N_BINS + KWIN], F32)
                            nc.vector.memset(cp_row[:], 0.0)
                            nc.vector.tensor_scalar(cp_row[:, KWIN // 2:KWIN // 2 + N_BINS],
                                                    cdf_sb[:], c0, rden[:],
                                                    mybir.AluOpType.subtract, mybir.AluOpType.mult)
                            nc.sync.dma_start(cpad_dram.ap(), cp_row[:])

                            # ---------- G table build ----------
                            rhs_cv = sm.tile([KWIN, 2 * N_BINS], F32)
                            nc.sync.dma_start(rhs_cv[:, 0:N_BINS],
                                              mk_ap(cpad_dram.ap(), 0, [[1, KWIN], [1, N_BINS]]))
                            nc.sync.dma_start(rhs_cv[:, N_BINS:2 * N_BINS],
                                              mk_ap(vpad_dram.ap(), 0, [[1, KWIN], [1, N_BINS]]))
                            g_psum = psp.tile([MSUB, 2 * N_BINS], F32)
                            nc.tensor.matmul(g_psum[:], ww[:], rhs_cv[:], start=True, stop=True)
                            rec_den = sm.tile([MSUB, N_BINS], F32)
                            nc.vector.reciprocal(rec_den[:], g_psum[:, N_BINS:2 * N_BINS])
                            g_sb = sm.tile([MSUB, N_BINS], F32)
                            nc.vector.tensor_tensor(g_sb[:], g_psum[:, 0:N_BINS], rec_den[:],
                                                    mybir.AluOpType.mult)
                            # gtab[16*q' + m] = g_sb[m, q']
                            nc.sync.dma_start(
                                mk_ap(gtab_dram.ap(), 0, [[1, MSUB], [MSUB, N_BINS]]), g_sb[:])

                            if stage == 2:
                                nc.sync.dma_start(
                                    out_dram.ap()[0:TLEN].rearrange("(a b) -> a b", a=MSUB),
                                    g_sb[:].transpose([1, 0]) if False else g_sb[:])
                            else:
                                # ---------- broadcast table to all partitions ----------
                                tab_sb = big.tile([128, TLEN], F32)
                                nc.sync.dma_start(
                                    tab_sb[:],
                                    gtab_dram.ap().rearrange("(a b) -> a b", a=1)
                                    .to_broadcast((128, TLEN)))

                                # ---------- phase 2: per-pixel lookup ----------
                                # idx directly from the contiguous x tile;
                                # round(x*TGRID): HW convert rounds-to-nearest
                                idx_sb = big.tile([128, NTILE], I16)
                                nc.vector.tensor_scalar(idx_sb[:], x_sb[:],
                                                        float(TGRID), None,
                                                        mybir.AluOpType.mult)

                                gout = big.tile([128, NPX_GRP], F32)
                                nc.gpsimd.ap_gather(
                                    gout[:].rearrange("c (n d) -> c n d", d=1),
                                    tab_sb[:].rearrange("c (n d) -> c n d", d=1),
                                    idx_sb[:], channels=128, num_elems=TLEN, d=1,
                                    num_idxs=NPX_GRP)

                                # gout[16g, 16s+r] holds pixel 8192g+512r+s.
                                # Store as-is (8 contiguous descriptors); the
                                # host unshard undoes the 16x512 wrap.
                                nc.sync.dma_start(
                                    out_dram.ap().rearrange("(a b) -> a b", a=8),
                                    gout[::16, :])
    nc.compile()
    return nc


_NC_CACHE = None


def _get_nc():
    global _NC_CACHE
    if _NC_CACHE is None:
        _NC_CACHE = build_nc()
    return _NC_CACHE


def _axon_device_reset():
    """Recover a wedged axon terminal (NRT_EXEC_UNIT_UNRECOVERABLE)."""
    try:
        import ctypes
        import jax
        jax.devices()
        lib = ctypes.CDLL("/opt/axon/libaxon_pjrt.so")
        if hasattr(lib, "axon_reset"):
            lib.axon_reset.restype = ctypes.c_int64
            lib.axon_reset()
    except Exception:
        pass


def kernel(x: np.ndarray) -> np.ndarray:
    assert x.shape == (B, 1, H, W), x.shape
    x = np.ascontiguousarray(np.asarray(x, dtype=np.float32))
    nc = _get_nc()
    in_maps = []
    for core in range(N_CORES):
        b, q = core // 4, core % 4
        shard = x[b, 0, q * 128:(q + 1) * 128, :].reshape(QUARTER)
        in_maps.append({"x": np.ascontiguousarray(shard)})
    try:
        res = run_bass_kernel_spmd(nc, in_maps, core_ids=list(range(N_CORES)))
    except Exception:
        _axon_device_reset()
        res = run_bass_kernel_spmd(nc, in_maps, core_ids=list(range(N_CORES)))
    out = np.empty((B, 1, H, W), np.float32)
    for core in range(N_CORES):
        b, q = core // 4, core % 4
        r = res.results[core]["out"].reshape(8, 512, 16).transpose(0, 2, 1)
        out[b, 0, q * 128:(q + 1) * 128, :] = r.reshape(128, W)
    return out

